# revision 6
# baseline (speedup 1.0000x reference)
"""Trainium2 Bass kernel for nn_Actor2ActorAttention (2-layer edge-attention GNN).

Strategy (single SPMD launch on 8 NeuronCores):
  - Host: sort edges by dst, partition dst range across 8 cores (125 dst per
    "window", 50 windows per core), pad each window to a fixed tile count,
    build int16 gather-index streams + per-edge scalar streams.
  - Device, per layer:
      dense phase: feat @ [W_src^T | w_att | W_dst^T] -> per-actor table
        rows [U | a_src | a_dst] (gatherable by edge src) + per-core-shard
        tables of a_dst and V (window-addressable).
      aggregation phase, per window: dma_gather the table rows of each
        edge's src (+ tiny a_dst gather by local dst), compute
        exp(leaky_relu(score)), build per-tile one-hot-times-ex selector
        matrices, and use the tensor engine to segment-reduce payload,
        denominator and rel-term moments in one PSUM accumulation.
        Finalize: agg = (P + qx*Wr0 + qy*Wr1)/denom + [denom>0]*V, relu.
  - AllGather (collective) exchanges layer-1 features between layers.
"""

import os
import sys

sys.path.insert(0, "/opt/trn_rl_repo")

import numpy as np

# ---------------- problem constants (hardcoded per spec) ----------------
N_ACTORS = 50000
D = 128
L = 2
E = 800000
NCORE = 8
DSTW = 125                      # dst actors per window
NWIN = 50                       # windows per core (8*50*125 == 50000)
SHARD = DSTW * NWIN             # 6250 dst actors per core
SHARD_PAD = 6272                # 49*128, keeps dense tiles shard-aligned
NPAD = NCORE * SHARD_PAD        # 50176 padded table rows
NDTILE = NPAD // 128            # 392 dense tiles
HALF = 32768                    # int16 gather split point
ROWE = 256                      # fp16 elements per tabU row (512B)

_PROGRAM_CACHE = {}


def _rho(a):
    """Actor id -> padded table row (shards padded to SHARD_PAD rows)."""
    return (a // SHARD) * SHARD_PAD + (a % SHARD)


def _wrap_idx(vals, n):
    """Lay out gather indices: element i at [i%16, i//16], replicated to all
    8 groups of 16 partitions -> [128, n//16] int16."""
    a = np.zeros(n, np.int16)
    a[: len(vals)] = vals
    return np.tile(a.reshape(-1, 16).T, (8, 1))


def _build_plan(src, dst, rel):
    src = np.asarray(src).astype(np.int64)
    dst = np.asarray(dst).astype(np.int64)
    rel = np.asarray(rel).astype(np.float32)

    order = np.argsort(dst, kind="stable")
    ss = _rho(src[order])
    sd = dst[order]
    sr = rel[order]

    g_of = sd // DSTW
    bounds = np.searchsorted(g_of, np.arange(NCORE * NWIN + 1))

    nlo = np.zeros(NCORE * NWIN, np.int64)
    nhi = np.zeros(NCORE * NWIN, np.int64)
    for g in range(NCORE * NWIN):
        s = ss[bounds[g]:bounds[g + 1]]
        nlo[g] = int((s < HALF).sum())
        nhi[g] = len(s) - nlo[g]
    TLO = max(1, int(np.ceil(nlo.max() / 128)))
    THI = max(1, int(np.ceil(nhi.max() / 128)))
    T = TLO + THI
    KW = (TLO + THI + T) * 8

    idxall = np.zeros((NCORE, NWIN, 128, KW), np.int16)
    rh = np.zeros((NCORE, NWIN, 128, T, 4), np.float16)
    dl = np.zeros((NCORE, NWIN, 128, T), np.float32)

    for g in range(NCORE * NWIN):
        c, w = divmod(g, NWIN)
        lo_e, hi_e = bounds[g], bounds[g + 1]
        s = ss[lo_e:hi_e]
        d_loc = (sd[lo_e:hi_e] - g * DSTW).astype(np.float32)
        r = sr[lo_e:hi_e]
        mlo = s < HALF

        # slot order: [lo edges, lo pads, hi edges, hi pads]
        n0, n1 = int(mlo.sum()), int((~mlo).sum())
        slots = T * 128
        sl_rel = np.zeros((slots, 2), np.float32)
        sl_dl = np.full(slots, 127.0, np.float32)
        sl_rel[:n0] = r[mlo]
        sl_dl[:n0] = d_loc[mlo]
        h0 = TLO * 128
        sl_rel[h0:h0 + n1] = r[~mlo]
        sl_dl[h0:h0 + n1] = d_loc[~mlo]

        idx_lo = s[mlo].astype(np.int16)
        idx_hi = (s[~mlo] - HALF).astype(np.int16)
        gd = np.zeros(slots, np.int16)
        gd[:n0] = (sd[lo_e:hi_e][mlo] - g * DSTW).astype(np.int16)
        gd[h0:h0 + n1] = (sd[lo_e:hi_e][~mlo] - g * DSTW).astype(np.int16)

        idxall[c, w, :, : TLO * 8] = _wrap_idx(idx_lo, TLO * 128)
        idxall[c, w, :, TLO * 8:(TLO + THI) * 8] = _wrap_idx(idx_hi, THI * 128)
        idxall[c, w, :, (TLO + THI) * 8:] = _wrap_idx(gd, T * 128)

        # slot i -> (partition i%128, tile i//128)
        rh[c, w, :, :, 0] = 1.0
        rh[c, w, :, :, 1] = sl_rel[:, 0].reshape(T, 128).T.astype(np.float16)
        rh[c, w, :, :, 2] = sl_rel[:, 1].reshape(T, 128).T.astype(np.float16)
        rh[c, w, :, :, 3] = sl_dl.reshape(T, 128).T.astype(np.float16)
        dl[c, w] = sl_dl.reshape(T, 128).T

    return T, TLO, THI, KW, idxall, rh, dl


def _build_program(T, TLO, THI, KW):
    key = (T, TLO, THI)
    if key in _PROGRAM_CACHE:
        return _PROGRAM_CACHE[key]

    import concourse.bass as bass
    import concourse.bacc as bacc
    import concourse.mybir as mybir
    import concourse.tile as tile

    f16 = mybir.dt.float16
    f32 = mybir.dt.float32
    i16 = mybir.dt.int16
    AF = mybir.ActivationFunctionType
    OP = mybir.AluOpType

    nc = bacc.Bacc("TRN2", target_bir_lowering=False, debug=False,
                   enable_asserts=True, num_devices=NCORE)

    F16 = nc.dram_tensor("feat0", [NPAD, D], f16, kind="ExternalInput").ap()
    IDX = nc.dram_tensor("idxall", [NWIN, 128, KW], i16, kind="ExternalInput").ap()
    RH = nc.dram_tensor("rh", [NWIN, 128, T, 4], f16, kind="ExternalInput").ap()
    DLOC = nc.dram_tensor("dloc", [NWIN, 128, T], f32, kind="ExternalInput").ap()
    WCAT = nc.dram_tensor("wcat", [L, 128, 260], f16, kind="ExternalInput").ap()
    WSC = nc.dram_tensor("wsc", [L, 128, 2], f32, kind="ExternalInput").ap()
    WRB = nc.dram_tensor("wrb", [L, 2, 128, 128], f16, kind="ExternalInput").ap()
    IOTA = nc.dram_tensor("iota", [128, 128], f16, kind="ExternalInput").ap()
    OUT = nc.dram_tensor("out", [SHARD, D], f32, kind="ExternalOutput").ap()

    tabU = nc.dram_tensor("tabU", [NPAD, ROWE], f16, kind="Internal").ap()
    tabAD = nc.dram_tensor("tabAD", [SHARD_PAD, 128], f16, kind="Internal").ap()
    tabV = nc.dram_tensor("tabV", [SHARD_PAD, 128], f16, kind="Internal").ap()
    f1own = nc.dram_tensor("f1own", [SHARD_PAD, D], f16, kind="Internal").ap()
    f1full = nc.dram_tensor("f1full", [NPAD, D], f16, kind="Internal",
                            addr_space="Shared").ap()

    with tile.TileContext(nc) as tc:
        with tc.tile_pool(name="const", bufs=1) as cp, \
             tc.tile_pool(name="dense", bufs=3) as dp, \
             tc.tile_pool(name="psumd", bufs=2, space="PSUM") as pdp, \
             tc.tile_pool(name="psumw", bufs=2, space="PSUM") as pwp, \
             tc.tile_pool(name="gath", bufs=2) as gp, \
             tc.tile_pool(name="sel", bufs=4) as selp, \
             tc.tile_pool(name="small", bufs=3) as sp, \
             tc.tile_pool(name="fin", bufs=2) as fp:

            iota_t = cp.tile([128, 128], f16, tag="iota")
            nc.sync.dma_start(iota_t[:], IOTA[:])
            wcat_t = [cp.tile([128, 260], f16, tag=f"wcat{l}", name=f"wcat{l}")
                      for l in range(L)]
            wsc_t = [cp.tile([128, 2], f32, tag=f"wsc{l}", name=f"wsc{l}")
                     for l in range(L)]
            wr0_t = [cp.tile([128, 128], f16, tag=f"wr0{l}", name=f"wr0{l}")
                     for l in range(L)]
            wr1_t = [cp.tile([128, 128], f16, tag=f"wr1{l}", name=f"wr1{l}")
                     for l in range(L)]
            for l in range(L):
                nc.sync.dma_start(wcat_t[l][:], WCAT[l])
                nc.sync.dma_start(wsc_t[l][:], WSC[l])
                nc.sync.dma_start(wr0_t[l][:], WRB[l, 0])
                nc.sync.dma_start(wr1_t[l][:], WRB[l, 1])

            # zero f1own pad rows once
            zt = cp.tile([128, 128], f16, tag="zt")
            nc.vector.memset(zt[:], 0.0)
            nc.sync.dma_start(f1own[SHARD:SHARD_PAD, :], zt[0:SHARD_PAD - SHARD, :])

            pid = nc.sync.partition_id()

            def dense_phase(l, featsrc):
                for i in range(NDTILE):
                    ftT = dp.tile([128, 128], f16, tag="ftT")
                    nc.sync.dma_start(ftT[:], featsrc[i * 128:(i + 1) * 128, :],
                                      transpose=True)
                    psd = pdp.tile([128, 260], f32, tag="psd")
                    nc.tensor.matmul(psd[:], ftT[:], wcat_t[l][:],
                                     start=True, stop=True)
                    stage = dp.tile([128, 130], f16, tag="stage")
                    nc.vector.tensor_copy(stage[:], psd[:, 0:130])
                    stageV = dp.tile([128, 128], f16, tag="stageV")
                    nc.scalar.activation(stageV[:], psd[:, 130:258], AF.Copy)
                    nc.sync.dma_start(tabU[i * 128:(i + 1) * 128, 0:130], stage[:])
                    owner = i // (SHARD_PAD // 128)
                    local = (i % (SHARD_PAD // 128)) * 128
                    nc.sync.dma_start(
                        tabAD[local:local + 128, 0:2], stage[:, 128:130],
                        cond=(pid == owner))
                    nc.sync.dma_start(
                        tabV[local:local + 128, :], stageV[:],
                        cond=(pid == owner))

            def agg_phase(l):
                for w in range(NWIN):
                    It = sp.tile([128, KW], i16, tag="It")
                    nc.sync.dma_start(It[:], IDX[w])
                    G = gp.tile([128, T * ROWE], f16, tag="G")
                    G3 = G[:].rearrange("p (t e) -> p t e", e=ROWE)
                    Gd = gp.tile([128, T * 128], f16, tag="Gd")
                    Gd3 = Gd[:].rearrange("p (t e) -> p t e", e=128)
                    nc.gpsimd.dma_gather(
                        out_ap=G3[:, 0:TLO, :], in_ap=tabU[0:HALF, :],
                        idxs_ap=It[:, 0:TLO * 8],
                        num_idxs=TLO * 128, num_idxs_reg=TLO * 128,
                        elem_size=ROWE, single_packet=False)
                    nc.gpsimd.dma_gather(
                        out_ap=G3[:, TLO:T, :], in_ap=tabU[HALF:NPAD, :],
                        idxs_ap=It[:, TLO * 8:(TLO + THI) * 8],
                        num_idxs=THI * 128, num_idxs_reg=THI * 128,
                        elem_size=ROWE, single_packet=False)
                    nc.gpsimd.dma_gather(
                        out_ap=Gd3[:, :, :],
                        in_ap=tabAD[w * DSTW:w * DSTW + DSTW, :],
                        idxs_ap=It[:, (TLO + THI) * 8:KW],
                        num_idxs=T * 128, num_idxs_reg=T * 128,
                        elem_size=128, single_packet=False)
                    # host streams (one, relx, rely, dloc) into pad cols of G
                    nc.sync.dma_start(G3[:, :, 130:134], RH[w])
                    DLt = sp.tile([128, T], f32, tag="DLt")
                    nc.sync.dma_start(DLt[:], DLOC[w])
                    Vw = fp.tile([128, 128], f16, tag="Vw")
                    nc.sync.dma_start(Vw[:], tabV[w * DSTW:w * DSTW + 128, :])

                    # scores [128, T] fp32
                    sA = sp.tile([128, T], f32, tag="sA")
                    sB = sp.tile([128, T], f32, tag="sB")
                    EX = sp.tile([128, T], f32, tag="EX")
                    nc.vector.tensor_scalar_mul(sA[:], G3[:, 0:T, 131],
                                                wsc_t[l][:, 0:1])
                    nc.gpsimd.tensor_scalar_mul(sB[:], G3[:, 0:T, 132],
                                                wsc_t[l][:, 1:2])
                    nc.vector.tensor_tensor(sA[:], sA[:], sB[:], OP.add)
                    nc.vector.tensor_tensor(sA[:], sA[:], G3[:, 0:T, 128], OP.add)
                    nc.vector.tensor_tensor(sA[:], sA[:], Gd3[:, 0:T, 1], OP.add)
                    nc.gpsimd.tensor_scalar_mul(sB[:], sA[:], 0.2)
                    nc.vector.tensor_tensor(sA[:], sA[:], sB[:], OP.max)
                    nc.scalar.activation(EX[:], sA[:], AF.Exp)

                    psW = pwp.tile([128, 134], f32, tag="psW")
                    for t in range(T):
                        St = selp.tile([128, 128], f16, tag="St")
                        eng = nc.vector if (t % 2 == 0) else nc.gpsimd
                        eng.tensor_scalar(
                            out=St[:], in0=iota_t[:],
                            scalar1=DLt[:, t:t + 1], scalar2=EX[:, t:t + 1],
                            op0=OP.is_equal, op1=OP.mult)
                        nc.tensor.matmul(psW[:], St[:], G3[:, t, 0:134],
                                         start=(t == 0), stop=(t == T - 1))

                    # finalize (gpsimd must not touch PSUM)
                    sc3 = sp.tile([128, 3], f32, tag="sc3")
                    nc.vector.tensor_copy(sc3[:], psW[:, 130:133])
                    den = sc3[:, 0:1]
                    qx = sc3[:, 1:2]
                    qy = sc3[:, 2:3]
                    c1 = sp.tile([128, 1], f32, tag="c1")
                    om = sp.tile([128, 1], f32, tag="om")
                    rc = sp.tile([128, 1], f32, tag="rc")
                    nc.gpsimd.tensor_scalar(out=c1[:], in0=den, scalar1=0.0,
                                            scalar2=None, op0=OP.is_gt)
                    nc.gpsimd.tensor_scalar(out=om[:], in0=c1[:], scalar1=-1.0,
                                            scalar2=1.0, op0=OP.mult, op1=OP.add)
                    nc.gpsimd.tensor_tensor(om[:], om[:], den, OP.add)
                    nc.vector.reciprocal(rc[:], om[:])
                    nc.vector.tensor_tensor(rc[:], rc[:], c1[:], OP.mult)
                    t0 = fp.tile([128, 128], f32, tag="t0")
                    t1 = fp.tile([128, 128], f32, tag="t1")
                    nc.vector.tensor_scalar_mul(t0[:], wr0_t[l][:], qx)
                    nc.gpsimd.tensor_scalar_mul(t1[:], wr1_t[l][:], qy)
                    nc.vector.tensor_tensor(t0[:], t0[:], t1[:], OP.add)
                    nc.vector.tensor_tensor(t0[:], t0[:], psW[:, 0:128], OP.add)
                    nc.vector.tensor_scalar_mul(t0[:], t0[:], rc[:])
                    nc.gpsimd.tensor_scalar_mul(t1[:], Vw[:], c1[:])
                    nc.vector.tensor_tensor(t0[:], t0[:], t1[:], OP.add)
                    if l == 0:
                        ot = fp.tile([128, 128], f16, tag="ot0")
                        nc.scalar.activation(ot[:], t0[:], AF.Relu)
                        nc.sync.dma_start(f1own[w * DSTW:w * DSTW + DSTW, :],
                                          ot[0:DSTW, :])
                    else:
                        ot = fp.tile([128, 128], f32, tag="ot1")
                        nc.scalar.activation(ot[:], t0[:], AF.Relu)
                        nc.sync.dma_start(OUT[w * DSTW:w * DSTW + DSTW, :],
                                          ot[0:DSTW, :])

            dense_phase(0, F16)
            tc.strict_bb_all_engine_barrier()
            agg_phase(0)
            tc.strict_bb_all_engine_barrier()
            nc.gpsimd.collective_compute(
                "AllGather", mybir.AluOpType.bypass,
                replica_groups=[list(range(NCORE))],
                ins=[f1own[:]], outs=[f1full[:]])
            tc.strict_bb_all_engine_barrier()
            dense_phase(1, f1full)
            tc.strict_bb_all_engine_barrier()
            agg_phase(1)

    nc.compile()
    _PROGRAM_CACHE[key] = nc
    return nc


def _host_inputs(inputs, T, TLO, THI, KW, idxall, rh, dl):
    af = np.asarray(inputs["actor_features"], np.float32)
    W_att = np.asarray(inputs["W_att"], np.float32)
    W_emb = np.asarray(inputs["W_emb"], np.float32)

    F16 = np.zeros((NPAD, D), np.float16)
    a = np.arange(N_ACTORS)
    F16[_rho(a)] = af.astype(np.float16)

    WCAT = np.zeros((L, 128, 260), np.float16)
    WSC = np.zeros((L, 128, 2), np.float32)
    WRB = np.zeros((L, 2, 128, 128), np.float16)
    for l in range(L):
        WCAT[l, :, 0:128] = W_emb[l][:, 0:128].T.astype(np.float16)
        WCAT[l, :, 128] = W_att[l][0:128].astype(np.float16)
        WCAT[l, :, 129] = W_att[l][130:258].astype(np.float16)
        WCAT[l, :, 130:258] = W_emb[l][:, 130:258].T.astype(np.float16)
        WSC[l, :, 0] = W_att[l][128]
        WSC[l, :, 1] = W_att[l][129]
        WRB[l, 0] = np.tile(W_emb[l][:, 128].astype(np.float16), (128, 1))
        WRB[l, 1] = np.tile(W_emb[l][:, 129].astype(np.float16), (128, 1))
    IOTA = np.tile(np.arange(128, dtype=np.float16), (128, 1))

    in_maps = []
    for c in range(NCORE):
        in_maps.append({
            "feat0": F16,
            "idxall": idxall[c],
            "rh": rh[c],
            "dloc": dl[c],
            "wcat": WCAT,
            "wsc": WSC,
            "wrb": WRB,
            "iota": IOTA,
        })
    return in_maps


def kernel(**inputs):
    from concourse import bass_utils

    T, TLO, THI, KW, idxall, rh, dl = _build_plan(
        inputs["edge_src_idx"], inputs["edge_dst_idx"], inputs["edge_dist_rel"])
    nc = _build_program(T, TLO, THI, KW)
    in_maps = _host_inputs(inputs, T, TLO, THI, KW, idxall, rh, dl)

    trace = os.environ.get("KERNEL_TRACE", "0") == "1"
    res = bass_utils.run_bass_kernel_spmd(
        nc, in_maps, core_ids=list(range(NCORE)), trace=trace)
    if trace and res.exec_time_ns is not None:
        print(f"HW exec time: {res.exec_time_ns} ns")

    out = np.concatenate([res.results[c]["out"] for c in range(NCORE)], axis=0)
    return out.astype(np.float32)


# revision 19
# speedup vs baseline: 1.7799x; 1.7799x over previous
"""Trainium2 Bass kernel for nn_Actor2ActorAttention (2-layer edge-attention GNN).

Strategy (single SPMD launch on 8 NeuronCores):
  - Host: sort edges by dst, partition dst range across 8 cores (125 dst per
    "window", 50 windows per core), pad each window to a fixed tile count,
    build int16 gather-index streams + per-edge scalar streams.
  - Device, per layer:
      dense phase: feat @ [W_src^T | w_att | W_dst^T] -> per-actor table
        rows [U | a_src | a_dst] (gatherable by edge src) + per-core-shard
        tables of a_dst and V (window-addressable).
      aggregation phase, per window: dma_gather the table rows of each
        edge's src (+ tiny a_dst gather by local dst), compute
        exp(leaky_relu(score)), build per-tile one-hot-times-ex selector
        matrices, and use the tensor engine to segment-reduce payload,
        denominator and rel-term moments in one PSUM accumulation.
        Finalize: agg = (P + qx*Wr0 + qy*Wr1)/denom + [denom>0]*V, relu.
  - AllGather (collective) exchanges layer-1 features between layers.
"""

import os
import sys

sys.path.insert(0, "/opt/trn_rl_repo")

import numpy as np

# ---------------- problem constants (hardcoded per spec) ----------------
N_ACTORS = 50000
D = 128
L = 2
E = 800000
NCORE = 8
DSTW = 125                      # dst actors per window
NWIN = 50                       # windows per core (8*50*125 == 50000)
SHARD = DSTW * NWIN             # 6250 dst actors per core
SHARD_PAD = 6656                # 13*512, keeps dense 4-tile chunks shard-aligned
NPAD = NCORE * SHARD_PAD        # 53248 padded table rows
NDTILE = NPAD // 128            # 416 dense tiles
DCH = 4                         # dense tiles per transpose-load chunk
HALF = 32768                    # int16 gather split point
ROWE = 256                      # fp16 elements per tabU row (512B)

_PROGRAM_CACHE = {}


def _rho(a):
    """Actor id -> padded table row (shards padded to SHARD_PAD rows)."""
    return (a // SHARD) * SHARD_PAD + (a % SHARD)


def _wrap_idx(vals, n):
    """Lay out gather indices: element i at [i%16, i//16], replicated to all
    8 groups of 16 partitions -> [128, n//16] int16."""
    a = np.zeros(n, np.int16)
    a[: len(vals)] = vals
    return np.tile(a.reshape(-1, 16).T, (8, 1))


def _build_plan(src, dst, rel):
    src = np.asarray(src).astype(np.int64)
    dst = np.asarray(dst).astype(np.int64)
    rel = np.asarray(rel).astype(np.float32)

    order = np.argsort(dst, kind="stable")
    ss = _rho(src[order])
    sd = dst[order]
    sr = rel[order]

    g_of = sd // DSTW
    bounds = np.searchsorted(g_of, np.arange(NCORE * NWIN + 1))

    nlo = np.zeros(NCORE * NWIN, np.int64)
    nhi = np.zeros(NCORE * NWIN, np.int64)
    for g in range(NCORE * NWIN):
        s = ss[bounds[g]:bounds[g + 1]]
        nlo[g] = int((s < HALF).sum())
        nhi[g] = len(s) - nlo[g]
    # per-window tile counts (max over cores for SPMD uniformity)
    TLOs = [max(1, int(np.ceil(nlo[w::NWIN].max() / 128))) for w in range(NWIN)]
    THIs = [max(1, int(np.ceil(nhi[w::NWIN].max() / 128))) for w in range(NWIN)]
    TLOs = tuple(TLOs)
    THIs = tuple(THIs)
    Tmax = max(lo + hi for lo, hi in zip(TLOs, THIs))
    KWmax = max((lo + hi) * 2 * 8 for lo, hi in zip(TLOs, THIs))

    idxall = np.zeros((NCORE, NWIN, 128, KWmax), np.int16)
    rh = np.zeros((NCORE, NWIN, 128, Tmax, 4), np.float16)
    dl = np.zeros((NCORE, NWIN, 128, Tmax), np.float16)

    for g in range(NCORE * NWIN):
        c, w = divmod(g, NWIN)
        TLO, THI = TLOs[w], THIs[w]
        T = TLO + THI
        lo_e, hi_e = bounds[g], bounds[g + 1]
        s = ss[lo_e:hi_e]
        d_loc = (sd[lo_e:hi_e] - g * DSTW).astype(np.float32)
        r = sr[lo_e:hi_e]
        mlo = s < HALF

        # slot order: [lo edges, lo pads, hi edges, hi pads]
        n0, n1 = int(mlo.sum()), int((~mlo).sum())
        slots = T * 128
        sl_rel = np.zeros((slots, 2), np.float32)
        sl_dl = np.full(slots, 127.0, np.float32)
        sl_rel[:n0] = r[mlo]
        sl_dl[:n0] = d_loc[mlo]
        h0 = TLO * 128
        sl_rel[h0:h0 + n1] = r[~mlo]
        sl_dl[h0:h0 + n1] = d_loc[~mlo]

        idx_lo = s[mlo].astype(np.int16)
        idx_hi = (s[~mlo] - HALF).astype(np.int16)
        gd = np.zeros(slots, np.int16)
        gd[:n0] = (sd[lo_e:hi_e][mlo] - g * DSTW).astype(np.int16)
        gd[h0:h0 + n1] = (sd[lo_e:hi_e][~mlo] - g * DSTW).astype(np.int16)

        idxall[c, w, :, : TLO * 8] = _wrap_idx(idx_lo, TLO * 128)
        idxall[c, w, :, TLO * 8:(TLO + THI) * 8] = _wrap_idx(idx_hi, THI * 128)
        idxall[c, w, :, (TLO + THI) * 8:(TLO + THI + T) * 8] = _wrap_idx(gd, T * 128)

        # slot i -> (partition i%128, tile i//128)
        rh[c, w, :, :T, 0] = 1.0
        rh[c, w, :, :T, 1] = sl_rel[:, 0].reshape(T, 128).T.astype(np.float16)
        rh[c, w, :, :T, 2] = sl_rel[:, 1].reshape(T, 128).T.astype(np.float16)
        rh[c, w, :, :T, 3] = sl_dl.reshape(T, 128).T.astype(np.float16)
        dl[c, w, :, :T] = sl_dl.reshape(T, 128).T

    return TLOs, THIs, Tmax, KWmax, idxall, rh, dl


def _build_program(TLOs, THIs, Tmax, KWmax):
    key = (TLOs, THIs)
    if key in _PROGRAM_CACHE:
        return _PROGRAM_CACHE[key]

    import concourse.bass as bass
    import concourse.bacc as bacc
    import concourse.mybir as mybir
    import concourse.tile as tile

    f16 = mybir.dt.float16
    f32 = mybir.dt.float32
    i16 = mybir.dt.int16
    AF = mybir.ActivationFunctionType
    OP = mybir.AluOpType

    nc = bacc.Bacc("TRN2", target_bir_lowering=False, debug=False,
                   enable_asserts=True, num_devices=NCORE, num_swdge_queues=4)

    F16 = nc.dram_tensor("feat0", [NPAD, D], f16, kind="ExternalInput").ap()
    IDX = nc.dram_tensor("idxall", [NWIN, 128, KWmax], i16, kind="ExternalInput").ap()
    RH = nc.dram_tensor("rh", [NWIN, 128, Tmax, 4], f16, kind="ExternalInput").ap()
    DLOC = nc.dram_tensor("dloc", [NWIN, 128, Tmax], f16, kind="ExternalInput").ap()
    IDENT = nc.dram_tensor("ident", [128, 128], f16, kind="ExternalInput").ap()
    WCAT = nc.dram_tensor("wcat", [L, 128, 260], f16, kind="ExternalInput").ap()
    WSC = nc.dram_tensor("wsc", [L, 128, 2], f32, kind="ExternalInput").ap()
    WRB = nc.dram_tensor("wrb", [L, 2, 128, 128], f16, kind="ExternalInput").ap()
    IOTA = nc.dram_tensor("iota", [128, 128], f16, kind="ExternalInput").ap()
    OUT = nc.dram_tensor("out", [SHARD, D], f32, kind="ExternalOutput").ap()

    tabU = nc.dram_tensor("tabU", [NPAD, ROWE], f16, kind="Internal").ap()
    tabAD = nc.dram_tensor("tabAD", [SHARD_PAD, 128], f16, kind="Internal").ap()
    tabV = nc.dram_tensor("tabV", [SHARD_PAD, 128], f16, kind="Internal").ap()
    f1own = nc.dram_tensor("f1own", [SHARD_PAD, D], f16, kind="Internal").ap()
    f1full = nc.dram_tensor("f1full", [NPAD, D], f16, kind="Internal",
                            addr_space="Shared").ap()

    with tile.TileContext(nc) as tc:
        with tc.tile_pool(name="const", bufs=1) as cp, \
             tc.tile_pool(name="dense", bufs=3) as dp, \
             tc.tile_pool(name="psumd", bufs=2, space="PSUM") as pdp, \
             tc.tile_pool(name="psumw", bufs=2, space="PSUM") as pwp, \
             tc.tile_pool(name="psumdt", bufs=2, space="PSUM") as pdt, \
             tc.tile_pool(name="psuma", bufs=2, space="PSUM") as pap, \
             tc.tile_pool(name="gath", bufs=4) as gp, \
             tc.tile_pool(name="sel", bufs=8) as selp, \
             tc.tile_pool(name="dall", bufs=3) as dallp, \
             tc.tile_pool(name="small", bufs=8) as sp, \
             tc.tile_pool(name="fin", bufs=4) as fp:

            iota_t = cp.tile([128, 128], f16, tag="iota")
            nc.sync.dma_start(iota_t[:], IOTA[:])
            ident_t = cp.tile([128, 128], f16, tag="ident")
            nc.sync.dma_start(ident_t[:], IDENT[:])
            wcat_t = [cp.tile([128, 260], f16, tag=f"wcat{l}", name=f"wcat{l}")
                      for l in range(L)]
            wsc_t = [cp.tile([128, 2], f32, tag=f"wsc{l}", name=f"wsc{l}")
                     for l in range(L)]
            wr0_t = [cp.tile([128, 128], f16, tag=f"wr0{l}", name=f"wr0{l}")
                     for l in range(L)]
            wr1_t = [cp.tile([128, 128], f16, tag=f"wr1{l}", name=f"wr1{l}")
                     for l in range(L)]
            for l in range(L):
                nc.sync.dma_start(wcat_t[l][:], WCAT[l])
                nc.sync.dma_start(wsc_t[l][:], WSC[l])
                nc.sync.dma_start(wr0_t[l][:], WRB[l, 0])
                nc.sync.dma_start(wr1_t[l][:], WRB[l, 1])

            # zero f1own pad rows once
            zt = cp.tile([128, 128], f16, tag="zt")
            nc.vector.memset(zt[:], 0.0)
            for zk in range(SHARD, SHARD_PAD, 128):
                zn = min(128, SHARD_PAD - zk)
                nc.sync.dma_start(f1own[zk:zk + zn, :], zt[0:zn, :])

            pid = nc.scalar.partition_id()

            def dense_phase(l, featsrc):
                for ic in range(NDTILE // DCH):
                    ftT = dp.tile([128, DCH * 128], f16, tag="ftT",
                                  name=f"ftT{l}_{ic}")
                    nc.sync.dma_start(ftT[:], featsrc[ic * DCH * 128:(ic + 1) * DCH * 128, :],
                                      transpose=True)
                    stg = dp.tile([128, DCH * 130], f16, tag="stg", name=f"stg{l}_{ic}")
                    stgV = dp.tile([128, DCH * 128], f16, tag="stgV", name=f"stgV{l}_{ic}")
                    for j in range(DCH):
                        psd = pdp.tile([128, 260], f32, tag="psd", name=f"psd{l}_{ic}_{j}")
                        nc.tensor.matmul(psd[:], ftT[:, j * 128:(j + 1) * 128],
                                         wcat_t[l][:], start=True, stop=True)
                        nc.vector.tensor_copy(stg[:, j * 130:(j + 1) * 130], psd[:, 0:130])
                        nc.scalar.activation(stgV[:, j * 128:(j + 1) * 128],
                                             psd[:, 130:258], AF.Copy)
                    stg3 = stg[:].rearrange("p (j e) -> p j e", e=130)
                    stgV3 = stgV[:].rearrange("p (j e) -> p j e", e=128)
                    outU = tabU[ic * DCH * 128:(ic + 1) * DCH * 128, 0:130]
                    nc.scalar.dma_start(outU.rearrange("(j p) e -> p j e", p=128), stg3)
                    owner = ic // (SHARD_PAD // (DCH * 128))
                    local = (ic % (SHARD_PAD // (DCH * 128))) * DCH * 128
                    outAD = tabAD[local:local + DCH * 128, 0:2]
                    nc.scalar.dma_start(outAD.rearrange("(j p) e -> p j e", p=128),
                                        stg3[:, :, 128:130], cond=(pid == owner))
                    outV = tabV[local:local + DCH * 128, :]
                    nc.scalar.dma_start(outV.rearrange("(j p) e -> p j e", p=128),
                                        stgV3, cond=(pid == owner))

            def agg_phase(l):
                for w in range(NWIN):
                    TLO, THI = TLOs[w], THIs[w]
                    T = TLO + THI
                    KW = 2 * T * 8
                    It = sp.tile([128, KW], i16, tag="It", name=f"It{l}_{w}")
                    nc.sync.dma_start(It[:, 0:(TLO + THI) * 8],
                                      IDX[w, :, 0:(TLO + THI) * 8])
                    G = gp.tile([128, T * ROWE], f16, tag="G", name=f"G{l}_{w}")
                    G3 = G[:].rearrange("p (t e) -> p t e", e=ROWE)
                    nc.gpsimd.dma_gather(
                        out_ap=G3[:, 0:TLO, :], in_ap=tabU[0:HALF, :],
                        idxs_ap=It[:, 0:TLO * 8],
                        num_idxs=TLO * 128, num_idxs_reg=TLO * 128,
                        elem_size=ROWE, single_packet=False,
                        queue_num=(2 * w) % 4)
                    nc.gpsimd.dma_gather(
                        out_ap=G3[:, TLO:T, :], in_ap=tabU[HALF:NPAD, :],
                        idxs_ap=It[:, TLO * 8:(TLO + THI) * 8],
                        num_idxs=THI * 128, num_idxs_reg=THI * 128,
                        elem_size=ROWE, single_packet=False,
                        queue_num=(2 * w + 1) % 4)
                    # host streams (one, relx, rely, dloc) into pad cols of G
                    nc.scalar.dma_start(G3[:, :, 130:134], RH[w, :, 0:T, :])
                    DLt = sp.tile([128, T], f16, tag="DLt", name=f"DL{l}_{w}")
                    nc.scalar.dma_start(DLt[:], DLOC[w, :, 0:T])
                    adw = sp.tile([128, 1], f16, tag="adw", name=f"adw{l}_{w}")
                    nc.scalar.dma_start(adw[:], tabAD[w * DSTW:w * DSTW + 128, 1:2])
                    Vw = fp.tile([128, 128], f16, tag="Vw", name=f"Vw{l}_{w}")
                    nc.scalar.dma_start(Vw[:], tabV[w * DSTW:w * DSTW + 128, :])

                    # one-hot D_t + a_dst expansion via PE transpose
                    psA = pap.tile([128, T], f32, tag="psA", name=f"psA{l}_{w}")
                    Dall = dallp.tile([128, T * 128], f16, tag="Dall",
                                     name=f"Dall{l}_{w}")
                    Dall3 = Dall[:].rearrange("p (t e) -> p t e", e=128)
                    for t in range(T):
                        nc.vector.tensor_tensor(
                            Dall3[:, t, :], iota_t[:],
                            DLt[:, t:t + 1].to_broadcast([128, 128]),
                            OP.is_equal)
                        psDT = pdt.tile([128, 128], f16, tag="psDT",
                                        name=f"psDT{l}_{w}_{t}")
                        nc.tensor.transpose(psDT[:], Dall3[:, t, :], ident_t[:])
                        DTs = selp.tile([128, 128], f16, tag="DTs",
                                        name=f"DTs{l}_{w}_{t}")
                        if t % 2 == 0:
                            nc.scalar.activation(DTs[:], psDT[:], AF.Copy)
                        else:
                            nc.vector.tensor_copy(DTs[:], psDT[:])
                        nc.tensor.matmul(psA[:, t:t + 1], DTs[:], adw[:],
                                         start=True, stop=True)

                    # scores [128, T] fp32
                    sA = sp.tile([128, T], f32, tag="sA", name=f"sA{l}_{w}")
                    sB = sp.tile([128, T], f32, tag="sB", name=f"sB{l}_{w}")
                    EX = sp.tile([128, T], f32, tag="EX", name=f"EX{l}_{w}")
                    nc.vector.tensor_tensor(
                        sA[:], G3[:, 0:T, 131], wsc_t[l][:, 0:1].to_broadcast([128, T]),
                        OP.mult)
                    nc.gpsimd.tensor_tensor(
                        sB[:], G3[:, 0:T, 132], wsc_t[l][:, 1:2].to_broadcast([128, T]),
                        OP.mult)
                    nc.vector.tensor_tensor(sA[:], sA[:], sB[:], OP.add)
                    nc.vector.tensor_tensor(sA[:], sA[:], G3[:, 0:T, 128], OP.add)
                    nc.vector.tensor_tensor(sA[:], sA[:], psA[:, 0:T], OP.add)
                    nc.gpsimd.tensor_scalar_mul(sB[:], sA[:], 0.2)
                    nc.vector.tensor_tensor(sA[:], sA[:], sB[:], OP.max)
                    nc.scalar.activation(EX[:], sA[:], AF.Exp)

                    psW = pwp.tile([128, 134], f32, tag="psW", name=f"psW{l}_{w}")
                    for t in range(T):
                        RS = selp.tile([128, 134], f16, tag="RS", name=f"RS{l}_{w}_{t}")
                        if t % 2 == 0:
                            nc.scalar.activation(RS[:], G3[:, t, 0:134], AF.Copy,
                                                 scale=EX[:, t:t + 1])
                        else:
                            nc.vector.tensor_tensor(
                                RS[:], G3[:, t, 0:134],
                                EX[:, t:t + 1].to_broadcast([128, 134]), OP.mult)
                        nc.tensor.matmul(psW[:], Dall3[:, t, :], RS[:],
                                         start=(t == 0), stop=(t == T - 1))

                    # finalize (gpsimd must not touch PSUM)
                    sc3 = sp.tile([128, 3], f32, tag="sc3", name=f"sc3{l}_{w}")
                    nc.vector.tensor_copy(sc3[:], psW[:, 130:133])
                    den = sc3[:, 0:1]
                    qx = sc3[:, 1:2]
                    qy = sc3[:, 2:3]
                    c1 = sp.tile([128, 1], f32, tag="c1", name=f"c1{l}_{w}")
                    om = sp.tile([128, 1], f32, tag="om", name=f"om{l}_{w}")
                    rc = sp.tile([128, 1], f32, tag="rc", name=f"rc{l}_{w}")
                    nc.gpsimd.tensor_scalar(out=c1[:], in0=den, scalar1=0.0,
                                            scalar2=None, op0=OP.is_gt)
                    nc.gpsimd.tensor_scalar(out=om[:], in0=c1[:], scalar1=-1.0,
                                            scalar2=1.0, op0=OP.mult, op1=OP.add)
                    nc.gpsimd.tensor_tensor(om[:], om[:], den, OP.add)
                    nc.vector.reciprocal(rc[:], om[:])
                    nc.vector.tensor_tensor(rc[:], rc[:], c1[:], OP.mult)
                    t0 = fp.tile([128, 128], f32, tag="t0", name=f"t0{l}_{w}")
                    t1 = fp.tile([128, 128], f32, tag="t1", name=f"t1{l}_{w}")
                    nc.vector.tensor_tensor(
                        t0[:], wr0_t[l][:], qx.to_broadcast([128, 128]), OP.mult)
                    nc.gpsimd.tensor_tensor(
                        t1[:], wr1_t[l][:], qy.to_broadcast([128, 128]), OP.mult)
                    nc.vector.tensor_tensor(t0[:], t0[:], t1[:], OP.add)
                    nc.vector.tensor_tensor(t0[:], t0[:], psW[:, 0:128], OP.add)
                    nc.vector.tensor_tensor(
                        t0[:], t0[:], rc[:].to_broadcast([128, 128]), OP.mult)
                    nc.gpsimd.tensor_tensor(
                        t1[:], Vw[:], c1[:].to_broadcast([128, 128]), OP.mult)
                    nc.vector.tensor_tensor(t0[:], t0[:], t1[:], OP.add)
                    if l == 0:
                        ot = fp.tile([128, 128], f16, tag="ot0", name=f"ot{l}_{w}")
                        nc.scalar.activation(ot[:], t0[:], AF.Relu)
                        nc.sync.dma_start(f1own[w * DSTW:w * DSTW + DSTW, :],
                                          ot[0:DSTW, :])
                    else:
                        ot = fp.tile([128, 128], f32, tag="ot1", name=f"ot{l}_{w}")
                        nc.scalar.activation(ot[:], t0[:], AF.Relu)
                        nc.sync.dma_start(OUT[w * DSTW:w * DSTW + DSTW, :],
                                          ot[0:DSTW, :])

            dense_phase(0, F16)
            tc.strict_bb_all_engine_barrier()
            agg_phase(0)
            tc.strict_bb_all_engine_barrier()
            nc.gpsimd.collective_compute(
                "AllGather", mybir.AluOpType.bypass,
                replica_groups=[list(range(NCORE))],
                ins=[f1own[:]], outs=[f1full[:]])
            tc.strict_bb_all_engine_barrier()
            dense_phase(1, f1full)
            tc.strict_bb_all_engine_barrier()
            agg_phase(1)

    nc.compile()
    _PROGRAM_CACHE[key] = nc
    return nc


def _host_inputs(inputs, idxall, rh, dl):
    af = np.asarray(inputs["actor_features"], np.float32)
    W_att = np.asarray(inputs["W_att"], np.float32)
    W_emb = np.asarray(inputs["W_emb"], np.float32)

    F16 = np.zeros((NPAD, D), np.float16)
    a = np.arange(N_ACTORS)
    F16[_rho(a)] = af.astype(np.float16)

    WCAT = np.zeros((L, 128, 260), np.float16)
    WSC = np.zeros((L, 128, 2), np.float32)
    WRB = np.zeros((L, 2, 128, 128), np.float16)
    for l in range(L):
        WCAT[l, :, 0:128] = W_emb[l][:, 0:128].T.astype(np.float16)
        WCAT[l, :, 128] = W_att[l][0:128].astype(np.float16)
        WCAT[l, :, 129] = W_att[l][130:258].astype(np.float16)
        WCAT[l, :, 130:258] = W_emb[l][:, 130:258].T.astype(np.float16)
        WSC[l, :, 0] = W_att[l][128]
        WSC[l, :, 1] = W_att[l][129]
        WRB[l, 0] = np.tile(W_emb[l][:, 128].astype(np.float16), (128, 1))
        WRB[l, 1] = np.tile(W_emb[l][:, 129].astype(np.float16), (128, 1))
    IOTA = np.tile(np.arange(128, dtype=np.float16), (128, 1))
    IDENT = np.eye(128, dtype=np.float16)

    in_maps = []
    for c in range(NCORE):
        in_maps.append({
            "feat0": F16,
            "idxall": idxall[c],
            "rh": rh[c],
            "dloc": dl[c],
            "wcat": WCAT,
            "wsc": WSC,
            "wrb": WRB,
            "iota": IOTA,
            "ident": IDENT,
        })
    return in_maps


def kernel(**inputs):
    from concourse import bass_utils

    TLOs, THIs, Tmax, KWmax, idxall, rh, dl = _build_plan(
        inputs["edge_src_idx"], inputs["edge_dst_idx"], inputs["edge_dist_rel"])
    nc = _build_program(TLOs, THIs, Tmax, KWmax)
    in_maps = _host_inputs(inputs, idxall, rh, dl)

    trace = os.environ.get("KERNEL_TRACE", "0") == "1"
    res = bass_utils.run_bass_kernel_spmd(
        nc, in_maps, core_ids=list(range(NCORE)), trace=trace)
    if trace and res.exec_time_ns is not None:
        print(f"HW exec time: {res.exec_time_ns} ns")

    out = np.concatenate([res.results[c]["out"] for c in range(NCORE)], axis=0)
    return out.astype(np.float32)
